# revision 25
# baseline (speedup 1.0000x reference)
"""CouncilCrossAttentionAdapter Trainium2 kernel.

Full inputs in, full output out. Data-parallel over 8 NeuronCores:
core c handles batch b = c//2, token half c%2 (2048 tokens each).

Algorithm (weights folded on host):
  u_j = hs @ (Wqj.T @ Wkj) / sqrt(H)       (same for i head)
  s_w = u . adv[p+w],  w in {0,1,2}        (window rows gathered on device)
  judge-head boolean surgery on s_j via per-token 3x3 coeff matrix
  a = softmax(s);  mix = sum_w a_w * adv[p+w]
  out = mix_j @ (w0*gain*Wvj.T@Wout.T) + mix_i @ (w1*gain*Wvi.T@Wout.T)
"""
import math
import os
import sys

sys.path.insert(0, "/opt/trn_rl_repo")

import numpy as np
import ml_dtypes

import concourse.bass as bass
import concourse.mybir as mybir
import concourse.tile as tile
from concourse import bacc
from concourse.bass_utils import run_bass_kernel_spmd
from concourse.masks import make_identity

B, T, L, H = 4, 4096, 4096, 1024
NCORES = 8
TOK = (B * T) // NCORES          # 2048 tokens per core
NTILES = TOK // 128              # 16
KC = H // 128                    # 8 k-chunks for the U matmul
KO = (2 * H) // 128              # 16 k-chunks for the out matmul
F32 = mybir.dt.float32
BF16 = mybir.dt.bfloat16
I32 = mybir.dt.int32

_cached = {}
MIXT_DMA = bool(int(os.environ.get("MIXT_DMA", "0")))


def _build_nc(reps=1):
    nc = bacc.Bacc("TRN2", target_bir_lowering=False, debug=False,
                   enable_asserts=False, num_devices=NCORES)

    hsp = nc.dram_tensor("hsp", [NTILES, 128, H], BF16, kind="ExternalInput")
    adv = nc.dram_tensor("adv", [L, H], F32, kind="ExternalInput")
    mcat = nc.dram_tensor("mcat", [128, KC, 2 * H], BF16, kind="ExternalInput")
    gcat = nc.dram_tensor("gcat", [128, KO, H], BF16, kind="ExternalInput")
    ptrt = nc.dram_tensor("ptrt", [NTILES, 128, 1], I32, kind="ExternalInput")
    coef = nc.dram_tensor("coef", [NTILES, 128, 9], F32, kind="ExternalInput")
    outt = nc.dram_tensor("outt", [TOK, H], F32, kind="ExternalOutput")

    with tile.TileContext(nc) as tc, \
         tc.tile_pool(name="wconst", bufs=1) as wpool, \
         tc.tile_pool(name="work", bufs=2) as work, \
         tc.tile_pool(name="scratch", bufs=2) as scratch, \
         tc.tile_pool(name="pu", bufs=2, space="PSUM") as pu_pool, \
         tc.tile_pool(name="pt", bufs=2, space="PSUM") as pt_pool, \
         tc.tile_pool(name="po", bufs=2, space="PSUM") as po_pool:

        ident = wpool.tile([128, 128], BF16, tag="ident")
        make_identity(nc, ident[:])

        # resident weights
        mc = []
        for c in range(KC):
            t = wpool.tile([128, 2 * H], BF16, tag=f"mc{c}")
            nc.sync.dma_start(out=t[:], in_=mcat[:, c, :])
            mc.append(t)
        gc = []
        for c in range(KO):
            t = wpool.tile([128, H], BF16, tag=f"gc{c}")
            nc.sync.dma_start(out=t[:], in_=gcat[:, c, :])
            gc.append(t)

        def tile_body():
          for i in range(NTILES):
            # ---- loads ----
            hs_sb = work.tile([128, H], BF16, tag="hs")       # [p=k%128, c*128+t]
            nc.sync.dma_start(out=hs_sb[:], in_=hsp[i])
            idx_sb = work.tile([128, 1], I32, tag="idx")
            nc.sync.dma_start(out=idx_sb[:], in_=ptrt[i])
            cf_sb = work.tile([128, 9], F32, tag="cf")
            nc.sync.dma_start(out=cf_sb[:], in_=coef[i])

            win = work.tile([128, 3 * H], F32, tag="win", bufs=3)  # [tok, w*H+h]
            nc.gpsimd.indirect_dma_start(
                out=win[:], out_offset=None,
                in_=adv[:, :],
                in_offset=bass.IndirectOffsetOnAxis(ap=idx_sb[:, :1], axis=0),
            )

            # ---- U = hs @ [Mj|Mi] : token-major [128, 2H] ----
            u_sb = work.tile([128, 2 * H], F32, tag="u")
            for s in range(4):
                pu = pu_pool.tile([128, 512], F32, tag="pu", name=f"pu{s}")
                for c in range(KC):
                    nc.tensor.matmul(pu[:], hs_sb[:, c * 128:(c + 1) * 128],
                                     mc[c][:, s * 512:(s + 1) * 512],
                                     start=(c == 0), stop=(c == KC - 1))
                nc.scalar.copy(u_sb[:, s * 512:(s + 1) * 512], pu[:])

            # ---- scores: s_hw = u_h . win_w ----
            sco = scratch.tile([128, 8], F32, tag="sco")      # [j0,j1,j2,?,i0,i1,i2,?]
            for h in range(2):
                pr3 = scratch.tile([128, 3 * H], F32, tag="pr3")
                uap = u_sb[:, h * H:(h + 1) * H]
                ub = bass.AP(uap.tensor, uap.offset, [uap.ap[0], [0, 3], uap.ap[1]])
                nc.gpsimd.tensor_tensor(
                    out=pr3[:].rearrange("p (w x) -> p w x", w=3),
                    in0=win[:].rearrange("p (w x) -> p w x", w=3),
                    in1=ub, op=mybir.AluOpType.mult)
                nc.vector.tensor_reduce(
                    out=sco[:, h * 4:h * 4 + 3],
                    in_=pr3[:].rearrange("p (w x) -> p w x", w=3),
                    axis=mybir.AxisListType.X, op=mybir.AluOpType.add)

            # ---- judge surgery: s_j0 = lin + max(u,v), [lin,u,v] = C @ [s0,s1,s2]
            sj_ap = sco[:, 0:3]
            sjb = bass.AP(sj_ap.tensor, sj_ap.offset,
                          [sj_ap.ap[0], [0, 3], sj_ap.ap[1]])
            prod = scratch.tile([128, 9], F32, tag="prod")
            nc.vector.tensor_tensor(out=prod[:].rearrange("p (g c) -> p g c", g=3),
                                    in0=sjb,
                                    in1=cf_sb[:].rearrange("p (g c) -> p g c", g=3),
                                    op=mybir.AluOpType.mult)
            guv = scratch.tile([128, 3], F32, tag="guv")
            nc.vector.tensor_reduce(out=guv[:], in_=prod[:].rearrange("p (g c) -> p g c", g=3),
                                    axis=mybir.AxisListType.X, op=mybir.AluOpType.add)
            nc.vector.tensor_tensor(out=sco[:, 0:1], in0=guv[:, 1:2], in1=guv[:, 2:3],
                                    op=mybir.AluOpType.max)
            nc.vector.tensor_tensor(out=sco[:, 0:1], in0=sco[:, 0:1], in1=guv[:, 0:1],
                                    op=mybir.AluOpType.add)

            # ---- softmax (no max-sub; scores are O(5)) ----
            ex = scratch.tile([128, 8], F32, tag="ex")
            den = scratch.tile([128, 2], F32, tag="den")
            rcp = scratch.tile([128, 2], F32, tag="rcp")
            for h in range(2):
                nc.scalar.activation(out=ex[:, h * 4:h * 4 + 3], in_=sco[:, h * 4:h * 4 + 3],
                                     func=mybir.ActivationFunctionType.Exp,
                                     accum_out=den[:, h:h + 1])
            nc.vector.reciprocal(rcp[:], den[:])
            for h in range(2):
                nc.vector.tensor_scalar_mul(ex[:, h * 4:h * 4 + 3], ex[:, h * 4:h * 4 + 3],
                                            rcp[:, h:h + 1])

            # ---- mix_h = sum_w a_hw * win_w  [128, 2H] bf16 ----
            mix = work.tile([128, 2 * H], BF16, tag="mix")
            for h in range(2):
                mt0 = scratch.tile([128, H], F32, tag="mt0")
                mt1 = scratch.tile([128, H], F32, tag="mt1")
                nc.scalar.activation(out=mt0[:], in_=win[:, 0:H],
                                     func=mybir.ActivationFunctionType.Copy,
                                     scale=ex[:, h * 4 + 0:h * 4 + 1])
                nc.scalar.activation(out=mt1[:], in_=win[:, H:2 * H],
                                     func=mybir.ActivationFunctionType.Copy,
                                     scale=ex[:, h * 4 + 1:h * 4 + 2])
                nc.vector.tensor_add(mt0[:], mt0[:], mt1[:])
                mt2 = scratch.tile([128, H], F32, tag="mt1", name="mt2")
                nc.scalar.activation(out=mt2[:], in_=win[:, 2 * H:3 * H],
                                     func=mybir.ActivationFunctionType.Copy,
                                     scale=ex[:, h * 4 + 2:h * 4 + 3])
                nc.vector.tensor_add(mix[:, h * H:(h + 1) * H], mt0[:], mt2[:])

            # ---- transpose mix -> mixT (k on partitions), bf16 ----
            mixT = work.tile([128, 2 * H], BF16, tag="mixT")
            if MIXT_DMA:
                # one xbar DMA transpose: mixT[p, c, t] = mix[t, c*128+p]
                nc.sync.dma_start_transpose(
                    out=mixT[:].rearrange("p (c t) -> p c t", c=KO), in_=mix[:])
            else:
                # 4 PE transposes share one PSUM bank, then a single batched evict
                for g in range(KO // 4):
                    pt = pt_pool.tile([128, 512], BF16, tag="pt")
                    for j in range(4):
                        c = g * 4 + j
                        nc.tensor.transpose(pt[:, j * 128:(j + 1) * 128],
                                            mix[:, c * 128:(c + 1) * 128], ident[:])
                    nc.scalar.copy(mixT[:, g * 512:(g + 1) * 512], pt[:])

            # ---- out = mixT.T @ Gcat ----
            po = po_pool.tile([128, H], F32, tag="po")
            for c in range(KO):
                lhs = mixT[:, c * 128:(c + 1) * 128]
                for s in range(2):
                    nc.tensor.matmul(po[:, s * 512:(s + 1) * 512], lhs,
                                     gc[c][:, s * 512:(s + 1) * 512],
                                     start=(c == 0), stop=(c == KO - 1))
            o_sb = work.tile([128, H], F32, tag="osb")
            nc.scalar.copy(o_sb[:, 0:512], po[:, 0:512])
            nc.vector.tensor_copy(o_sb[:, 512:H], po[:, 512:H])
            nc.sync.dma_start(out=outt[i * 128:(i + 1) * 128, :], in_=o_sb[:])

        if reps == 1:
            tile_body()
        else:
            with tc.For_i(0, reps, 1):
                tile_body()

    nc.compile()
    return nc


# surgery coefficient table: id -> [group(lin,u,v)][comp(l_rel,l1,l2)]
_CTAB = np.zeros((8, 3, 3), np.float32)
_CTAB[0, 0] = [0, 1, 1]
_CTAB[1, 1] = [0, 1, 0]; _CTAB[1, 2] = [0, 0, 1]
_CTAB[2, 0] = [0, -1, 0]
_CTAB[3, 1] = [0, -1, 0]; _CTAB[3, 2] = [0, 0, 1]
_CTAB[4, 1] = [0, 1, -1]; _CTAB[4, 2] = [0, -1, 1]
_CTAB[5, 0] = [1, 0, 0]; _CTAB[6, 0] = [1, 0, 0]; _CTAB[7, 0] = [1, 0, 0]


def kernel(hidden_states, advisor_states, advisor_ids, pointer_ids,
           Wqj, Wkj, Wvj, Wqi, Wki, Wvi, Wout, gain, council_weights,
           _trace=False):
    hs = np.ascontiguousarray(np.asarray(hidden_states, np.float32))
    adv = np.ascontiguousarray(np.asarray(advisor_states, np.float32))
    aid = np.asarray(advisor_ids)
    ptr = np.asarray(pointer_ids).astype(np.int64)
    gain_f = float(np.asarray(gain))
    cw = np.asarray(council_weights, np.float64)
    w = np.exp(cw - cw.max()); w = w / w.sum()
    inv = 1.0 / math.sqrt(H)

    f64 = np.float64
    Mj = np.asarray(Wqj, f64).T @ np.asarray(Wkj, f64) * inv
    Mi = np.asarray(Wqi, f64).T @ np.asarray(Wki, f64) * inv
    Gj = w[0] * gain_f * (np.asarray(Wvj, f64).T @ np.asarray(Wout, f64).T)
    Gi = w[1] * gain_f * (np.asarray(Wvi, f64).T @ np.asarray(Wout, f64).T)
    Mcat = np.concatenate([Mj, Mi], axis=1).astype(ml_dtypes.bfloat16)  # [H, 2H]
    Gcat = np.concatenate([Gj, Gi], axis=0).astype(ml_dtypes.bfloat16)  # [2H, H]

    # [p, c, n] packings so each SBUF partition row is contiguous in DRAM
    mcat_p = np.ascontiguousarray(Mcat.reshape(KC, 128, 2 * H).transpose(1, 0, 2))
    gcat_p = np.ascontiguousarray(Gcat.reshape(KO, 128, H).transpose(1, 0, 2))

    p_clip = np.clip(ptr, 0, L - 3)
    rel = np.take_along_axis(aid, p_clip, axis=1)   # [B,T]
    coef_full = _CTAB[rel].reshape(B, T, 9)

    in_maps = []
    for core in range(NCORES):
        b, half = core // 2, core % 2
        sl = slice(half * TOK, (half + 1) * TOK)
        hsc = hs[b, sl].astype(ml_dtypes.bfloat16)           # [TOK, H]
        # hsp[i, p, c*128+t] = hs[i*128+t, c*128+p]
        hsp = np.ascontiguousarray(
            hsc.reshape(NTILES, 128, KC, 128).transpose(0, 3, 2, 1)
        ).reshape(NTILES, 128, H)
        in_maps.append({
            "hsp": hsp,
            "adv": adv[b],
            "mcat": mcat_p,
            "gcat": gcat_p,
            "ptrt": np.ascontiguousarray(
                p_clip[b, sl].astype(np.int32).reshape(NTILES, 128, 1)),
            "coef": np.ascontiguousarray(
                coef_full[b, sl].reshape(NTILES, 128, 9).astype(np.float32)),
        })

    if "nc" not in _cached:
        _cached["nc"] = _build_nc()
    nc = _cached["nc"]

    res = run_bass_kernel_spmd(nc, in_maps, list(range(NCORES)), trace=_trace)
    outs = [res.results[c]["outt"] for c in range(NCORES)]
    out = np.empty((B, T, H), np.float32)
    for core in range(NCORES):
        b, half = core // 2, core % 2
        out[b, half * TOK:(half + 1) * TOK] = outs[core]
    if _trace:
        kernel._last = res
    return out


# revision 28
# speedup vs baseline: 101.2588x; 101.2588x over previous
"""CouncilCrossAttentionAdapter Trainium2 kernel.

Full inputs in, full output out. Data-parallel over 8 NeuronCores:
core c handles batch b = c//2, token half c%2 (2048 tokens each).

Algorithm (weights folded on host):
  u_j = hs @ (Wqj.T @ Wkj) / sqrt(H)       (same for i head)
  s_w = u . adv[p+w],  w in {0,1,2}        (window rows gathered on device)
  judge-head boolean surgery on s_j via per-token 3x3 coeff matrix
  a = softmax(s);  mix = sum_w a_w * adv[p+w]
  out = mix_j @ (w0*gain*Wvj.T@Wout.T) + mix_i @ (w1*gain*Wvi.T@Wout.T)
"""
import math
import os
import sys

sys.path.insert(0, "/opt/trn_rl_repo")

import numpy as np
import ml_dtypes

import concourse.bass as bass
import concourse.mybir as mybir
import concourse.tile as tile
from concourse import bacc
from concourse.bass_utils import run_bass_kernel_spmd
from concourse.masks import make_identity

B, T, L, H = 4, 4096, 4096, 1024
NCORES = 8
TOK = (B * T) // NCORES          # 2048 tokens per core
NTILES = TOK // 128              # 16
KC = H // 128                    # 8 k-chunks for the U matmul
KO = (2 * H) // 128              # 16 k-chunks for the out matmul
F32 = mybir.dt.float32
BF16 = mybir.dt.bfloat16
I32 = mybir.dt.int32

_cached = {}
MIXT_DMA = bool(int(os.environ.get("MIXT_DMA", "0")))
SKIP_GATHER = bool(int(os.environ.get("SKIP_GATHER", "0")))
POOL_MUL = bool(int(os.environ.get("POOL_MUL", "1")))
SKIP_COMPUTE = bool(int(os.environ.get("SKIP_COMPUTE", "0")))


def _build_nc(reps=1):
    nc = bacc.Bacc("TRN2", target_bir_lowering=False, debug=False,
                   enable_asserts=False, num_devices=NCORES)

    hsp = nc.dram_tensor("hsp", [NTILES, 128, H], BF16, kind="ExternalInput")
    adv = nc.dram_tensor("adv", [L, H], F32, kind="ExternalInput")
    mcat = nc.dram_tensor("mcat", [128, KC, 2 * H], BF16, kind="ExternalInput")
    gcat = nc.dram_tensor("gcat", [128, KO, H], BF16, kind="ExternalInput")
    ptrt = nc.dram_tensor("ptrt", [NTILES, 128, 1], I32, kind="ExternalInput")
    coef = nc.dram_tensor("coef", [NTILES, 128, 9], F32, kind="ExternalInput")
    outt = nc.dram_tensor("outt", [TOK, H], F32, kind="ExternalOutput")

    with tile.TileContext(nc) as tc, \
         tc.tile_pool(name="wconst", bufs=1) as wpool, \
         tc.tile_pool(name="work", bufs=2) as work, \
         tc.tile_pool(name="scratch", bufs=2) as scratch, \
         tc.tile_pool(name="pu", bufs=2, space="PSUM") as pu_pool, \
         tc.tile_pool(name="pt", bufs=2, space="PSUM") as pt_pool, \
         tc.tile_pool(name="po", bufs=2, space="PSUM") as po_pool:

        ident = wpool.tile([128, 128], BF16, tag="ident")
        make_identity(nc, ident[:])

        # resident weights
        mc = []
        for c in range(KC):
            t = wpool.tile([128, 2 * H], BF16, tag=f"mc{c}")
            nc.sync.dma_start(out=t[:], in_=mcat[:, c, :])
            mc.append(t)
        gc = []
        for c in range(KO):
            t = wpool.tile([128, H], BF16, tag=f"gc{c}")
            nc.sync.dma_start(out=t[:], in_=gcat[:, c, :])
            gc.append(t)

        def tile_body():
          for i in range(NTILES):
            # ---- loads ----
            hs_sb = work.tile([128, H], BF16, tag="hs")       # [p=k%128, c*128+t]
            nc.sync.dma_start(out=hs_sb[:], in_=hsp[i])
            idx_sb = work.tile([128, 1], I32, tag="idx")
            nc.sync.dma_start(out=idx_sb[:], in_=ptrt[i])
            cf_sb = work.tile([128, 9], F32, tag="cf")
            nc.sync.dma_start(out=cf_sb[:], in_=coef[i])

            win = work.tile([128, 3 * H], F32, tag="win", bufs=3)  # [tok, w*H+h]
            if not SKIP_GATHER:
                nc.gpsimd.indirect_dma_start(
                    out=win[:], out_offset=None,
                    in_=adv[:, :],
                    in_offset=bass.IndirectOffsetOnAxis(ap=idx_sb[:, :1], axis=0),
                )
            else:
                nc.sync.dma_start(out=win[:, 0:H], in_=adv[i * 128:(i + 1) * 128, :])
                nc.sync.dma_start(out=win[:, H:2 * H], in_=adv[i * 128:(i + 1) * 128, :])
                nc.sync.dma_start(out=win[:, 2 * H:3 * H], in_=adv[i * 128:(i + 1) * 128, :])

            if SKIP_COMPUTE:
                o_sb0 = work.tile([128, H], F32, tag="osb", name="o_sb0")
                nc.vector.tensor_copy(o_sb0[:], win[:, 0:H])
                nc.sync.dma_start(out=outt[i * 128:(i + 1) * 128, :], in_=o_sb0[:])
                continue

            # ---- U = hs @ [Mj|Mi] : token-major [128, 2H] ----
            u_sb = work.tile([128, 2 * H], F32, tag="u")
            for s in range(4):
                pu = pu_pool.tile([128, 512], F32, tag="pu", name=f"pu{s}")
                for c in range(KC):
                    nc.tensor.matmul(pu[:], hs_sb[:, c * 128:(c + 1) * 128],
                                     mc[c][:, s * 512:(s + 1) * 512],
                                     start=(c == 0), stop=(c == KC - 1))
                nc.scalar.copy(u_sb[:, s * 512:(s + 1) * 512], pu[:])

            # ---- scores: s_hw = u_h . win_w ----
            sco = scratch.tile([128, 8], F32, tag="sco")      # [j0,j1,j2,?,i0,i1,i2,?]
            for h in range(2):
                pr3 = scratch.tile([128, 3 * H], F32, tag="pr3")
                uap = u_sb[:, h * H:(h + 1) * H]
                ub = bass.AP(uap.tensor, uap.offset, [uap.ap[0], [0, 3], uap.ap[1]])
                eng = nc.gpsimd if POOL_MUL else nc.vector
                eng.tensor_tensor(
                    out=pr3[:].rearrange("p (w x) -> p w x", w=3),
                    in0=win[:].rearrange("p (w x) -> p w x", w=3),
                    in1=ub, op=mybir.AluOpType.mult)
                nc.vector.tensor_reduce(
                    out=sco[:, h * 4:h * 4 + 3],
                    in_=pr3[:].rearrange("p (w x) -> p w x", w=3),
                    axis=mybir.AxisListType.X, op=mybir.AluOpType.add)

            # ---- judge surgery: s_j0 = lin + max(u,v), [lin,u,v] = C @ [s0,s1,s2]
            sj_ap = sco[:, 0:3]
            sjb = bass.AP(sj_ap.tensor, sj_ap.offset,
                          [sj_ap.ap[0], [0, 3], sj_ap.ap[1]])
            prod = scratch.tile([128, 9], F32, tag="prod")
            nc.vector.tensor_tensor(out=prod[:].rearrange("p (g c) -> p g c", g=3),
                                    in0=sjb,
                                    in1=cf_sb[:].rearrange("p (g c) -> p g c", g=3),
                                    op=mybir.AluOpType.mult)
            guv = scratch.tile([128, 3], F32, tag="guv")
            nc.vector.tensor_reduce(out=guv[:], in_=prod[:].rearrange("p (g c) -> p g c", g=3),
                                    axis=mybir.AxisListType.X, op=mybir.AluOpType.add)
            nc.vector.tensor_tensor(out=sco[:, 0:1], in0=guv[:, 1:2], in1=guv[:, 2:3],
                                    op=mybir.AluOpType.max)
            nc.vector.tensor_tensor(out=sco[:, 0:1], in0=sco[:, 0:1], in1=guv[:, 0:1],
                                    op=mybir.AluOpType.add)

            # ---- softmax (no max-sub; scores are O(5)) ----
            ex = scratch.tile([128, 8], F32, tag="ex")
            den = scratch.tile([128, 2], F32, tag="den")
            rcp = scratch.tile([128, 2], F32, tag="rcp")
            for h in range(2):
                nc.scalar.activation(out=ex[:, h * 4:h * 4 + 3], in_=sco[:, h * 4:h * 4 + 3],
                                     func=mybir.ActivationFunctionType.Exp,
                                     accum_out=den[:, h:h + 1])
            nc.vector.reciprocal(rcp[:], den[:])
            for h in range(2):
                nc.vector.tensor_scalar_mul(ex[:, h * 4:h * 4 + 3], ex[:, h * 4:h * 4 + 3],
                                            rcp[:, h:h + 1])

            # ---- mix_h = sum_w a_hw * win_w  [128, 2H] bf16 ----
            mix = work.tile([128, 2 * H], BF16, tag="mix")
            for h in range(2):
                mt0 = scratch.tile([128, H], F32, tag="mt0")
                mt1 = scratch.tile([128, H], F32, tag="mt1")
                nc.scalar.activation(out=mt0[:], in_=win[:, 0:H],
                                     func=mybir.ActivationFunctionType.Copy,
                                     scale=ex[:, h * 4 + 0:h * 4 + 1])
                nc.scalar.activation(out=mt1[:], in_=win[:, H:2 * H],
                                     func=mybir.ActivationFunctionType.Copy,
                                     scale=ex[:, h * 4 + 1:h * 4 + 2])
                nc.vector.tensor_add(mt0[:], mt0[:], mt1[:])
                mt2 = scratch.tile([128, H], F32, tag="mt1", name="mt2")
                nc.scalar.activation(out=mt2[:], in_=win[:, 2 * H:3 * H],
                                     func=mybir.ActivationFunctionType.Copy,
                                     scale=ex[:, h * 4 + 2:h * 4 + 3])
                nc.vector.tensor_add(mix[:, h * H:(h + 1) * H], mt0[:], mt2[:])

            # ---- transpose mix -> mixT (k on partitions), bf16 ----
            mixT = work.tile([128, 2 * H], BF16, tag="mixT")
            if MIXT_DMA:
                # one xbar DMA transpose: mixT[p, c, t] = mix[t, c*128+p]
                nc.sync.dma_start_transpose(
                    out=mixT[:].rearrange("p (c t) -> p c t", c=KO), in_=mix[:])
            else:
                # 4 PE transposes share one PSUM bank, then a single batched evict
                for g in range(KO // 4):
                    pt = pt_pool.tile([128, 512], BF16, tag="pt")
                    for j in range(4):
                        c = g * 4 + j
                        nc.tensor.transpose(pt[:, j * 128:(j + 1) * 128],
                                            mix[:, c * 128:(c + 1) * 128], ident[:])
                    nc.scalar.copy(mixT[:, g * 512:(g + 1) * 512], pt[:])

            # ---- out = mixT.T @ Gcat ----
            po = po_pool.tile([128, H], F32, tag="po")
            for c in range(KO):
                lhs = mixT[:, c * 128:(c + 1) * 128]
                for s in range(2):
                    nc.tensor.matmul(po[:, s * 512:(s + 1) * 512], lhs,
                                     gc[c][:, s * 512:(s + 1) * 512],
                                     start=(c == 0), stop=(c == KO - 1))
            o_sb = work.tile([128, H], F32, tag="osb")
            nc.scalar.copy(o_sb[:, 0:512], po[:, 0:512])
            nc.vector.tensor_copy(o_sb[:, 512:H], po[:, 512:H])
            nc.sync.dma_start(out=outt[i * 128:(i + 1) * 128, :], in_=o_sb[:])

        if reps == 1:
            tile_body()
        else:
            with tc.For_i(0, reps, 1):
                tile_body()

    nc.compile()
    return nc


# surgery coefficient table: id -> [group(lin,u,v)][comp(l_rel,l1,l2)]
_CTAB = np.zeros((8, 3, 3), np.float32)
_CTAB[0, 0] = [0, 1, 1]
_CTAB[1, 1] = [0, 1, 0]; _CTAB[1, 2] = [0, 0, 1]
_CTAB[2, 0] = [0, -1, 0]
_CTAB[3, 1] = [0, -1, 0]; _CTAB[3, 2] = [0, 0, 1]
_CTAB[4, 1] = [0, 1, -1]; _CTAB[4, 2] = [0, -1, 1]
_CTAB[5, 0] = [1, 0, 0]; _CTAB[6, 0] = [1, 0, 0]; _CTAB[7, 0] = [1, 0, 0]


def kernel(hidden_states, advisor_states, advisor_ids, pointer_ids,
           Wqj, Wkj, Wvj, Wqi, Wki, Wvi, Wout, gain, council_weights,
           _trace=False):
    hs = np.ascontiguousarray(np.asarray(hidden_states, np.float32))
    adv = np.ascontiguousarray(np.asarray(advisor_states, np.float32))
    aid = np.asarray(advisor_ids)
    ptr = np.asarray(pointer_ids).astype(np.int64)
    gain_f = float(np.asarray(gain))
    cw = np.asarray(council_weights, np.float64)
    w = np.exp(cw - cw.max()); w = w / w.sum()
    inv = 1.0 / math.sqrt(H)

    f64 = np.float64
    Mj = np.asarray(Wqj, f64).T @ np.asarray(Wkj, f64) * inv
    Mi = np.asarray(Wqi, f64).T @ np.asarray(Wki, f64) * inv
    Gj = w[0] * gain_f * (np.asarray(Wvj, f64).T @ np.asarray(Wout, f64).T)
    Gi = w[1] * gain_f * (np.asarray(Wvi, f64).T @ np.asarray(Wout, f64).T)
    Mcat = np.concatenate([Mj, Mi], axis=1).astype(ml_dtypes.bfloat16)  # [H, 2H]
    Gcat = np.concatenate([Gj, Gi], axis=0).astype(ml_dtypes.bfloat16)  # [2H, H]

    # [p, c, n] packings so each SBUF partition row is contiguous in DRAM
    mcat_p = np.ascontiguousarray(Mcat.reshape(KC, 128, 2 * H).transpose(1, 0, 2))
    gcat_p = np.ascontiguousarray(Gcat.reshape(KO, 128, H).transpose(1, 0, 2))

    p_clip = np.clip(ptr, 0, L - 3)
    rel = np.take_along_axis(aid, p_clip, axis=1)   # [B,T]
    coef_full = _CTAB[rel].reshape(B, T, 9)

    in_maps = []
    for core in range(NCORES):
        b, half = core // 2, core % 2
        sl = slice(half * TOK, (half + 1) * TOK)
        hsc = hs[b, sl].astype(ml_dtypes.bfloat16)           # [TOK, H]
        # hsp[i, p, c*128+t] = hs[i*128+t, c*128+p]
        hsp = np.ascontiguousarray(
            hsc.reshape(NTILES, 128, KC, 128).transpose(0, 3, 2, 1)
        ).reshape(NTILES, 128, H)
        in_maps.append({
            "hsp": hsp,
            "adv": adv[b],
            "mcat": mcat_p,
            "gcat": gcat_p,
            "ptrt": np.ascontiguousarray(
                p_clip[b, sl].astype(np.int32).reshape(NTILES, 128, 1)),
            "coef": np.ascontiguousarray(
                coef_full[b, sl].reshape(NTILES, 128, 9).astype(np.float32)),
        })

    if "nc" not in _cached:
        _cached["nc"] = _build_nc()
    nc = _cached["nc"]

    res = run_bass_kernel_spmd(nc, in_maps, list(range(NCORES)), trace=_trace)
    outs = [res.results[c]["outt"] for c in range(NCORES)]
    out = np.empty((B, T, H), np.float32)
    for core in range(NCORES):
        b, half = core // 2, core % 2
        out[b, half * TOK:(half + 1) * TOK] = outs[core]
    if _trace:
        kernel._last = res
    return out
